# revision 20
# baseline (speedup 1.0000x reference)
"""Trainium2 Bass kernel for nn_DHHPTransform.

The reference op is: optional stride-2 permutation along N, an upper
tridiagonal Givens sweep, a lower tridiagonal sweep, and a diagonal
scale.  The two sweeps compose into a single *pentadiagonal* operator
  z[i] = sum_{k=-2..2} c_k[i] * x[i+k]
whose coefficients c_k (and the Diag fold) are O(B*N) and precomputed
on host.  The device kernel is a banded matvec: for each 128-row input
window one bf16 matmul  out[124, 256] = lhsT[128, 124].T @ win, PSUM
evicted to SBUF bf16, grouped contiguous stores.

All device-side IO is CONTIGUOUS in bf16:
 - x is pre-permuted on host (sweep-row order), so window loads are
   plain slices;
 - the banded slabs are host-baked k-major [KWIN, nslot*KWIN] so chunk
   loads are per-partition contiguous;
 - z is stored in a blocked [p][block][d] layout (one linear range per
   store group) and unscrambled on host.

Sharding: pure data-parallel, one batch element per NeuronCore.
"""

import numpy as np

B, N, D = 8, 8192, 256
KWIN = 128           # matmul contraction window (input rows per block)
MOUT = KWIN - 4      # output rows per block (window = out rows +2 halo each side)
NCORES = 8

# tunables; _get_program cache key includes them
CFG = {"XCH": 22, "LCH": 17, "GH": 11, "store_eng": "gpsimd", "psum_bufs": 6,
       "xg_bufs": 3, "stage_bufs": 4, "lh_bufs": 3, "swq": 1, "evr": 3,
       "dt": "bf16", "ssplit": 1, "smix": "p", "gplan": (), "xplan": (),
       "pfirst": 0, "lh_eng": "gpsimd"}

_prog_cache = {}


def _np_dt():
    if CFG["dt"] == "f32":
        return np.float32
    import ml_dtypes
    return ml_dtypes.bfloat16


# ---------------------------------------------------------------- host math

def _penta_coeffs(G_l_ii, G_l_ij, G_l_ji, G_l_jj,
                  G_u_ii, G_u_ij, G_u_ji, G_u_jj, Diag, transform):
    """[B, 5, N] pentadiagonal coefficients; index k means offset k-2."""
    Bn, n = Diag.shape
    f8 = np.float64
    u_lo = np.zeros((Bn, n), f8); u_dm = np.zeros((Bn, n), f8); u_hi = np.zeros((Bn, n), f8)
    u_dm[:, 0] = G_u_ii[:, 0]
    u_hi[:, 0] = G_u_ij[:, 0]
    u_lo[:, 1:n-1] = G_u_ji[:, :-1]
    u_dm[:, 1:n-1] = G_u_jj[:, :-1].astype(f8) * G_u_ii[:, 1:]
    u_hi[:, 1:n-1] = G_u_jj[:, :-1].astype(f8) * G_u_ij[:, 1:]
    u_lo[:, n-1] = G_u_ji[:, n-2]
    u_dm[:, n-1] = G_u_jj[:, n-2]
    l_lo = np.zeros((Bn, n), f8); l_dm = np.zeros((Bn, n), f8); l_hi = np.zeros((Bn, n), f8)
    l_dm[:, 0] = G_l_ii[:, 0]
    l_hi[:, 0] = G_l_ij[:, 0]
    l_lo[:, 1:n-1] = G_l_ii[:, 1:n-1].astype(f8) * G_l_ji[:, :n-2]
    l_dm[:, 1:n-1] = G_l_ii[:, 1:n-1].astype(f8) * G_l_jj[:, :n-2]
    l_hi[:, 1:n-1] = G_l_ij[:, 1:n-1]
    l_lo[:, n-1] = G_l_ji[:, n-2]
    l_dm[:, n-1] = G_l_jj[:, n-2]

    def sh(a, k):
        out = np.zeros_like(a)
        if k == 0:
            return a.copy()
        if k > 0:
            out[:, :-k] = a[:, k:]
        else:
            out[:, -k:] = a[:, :k]
        return out

    c = np.zeros((Bn, 5, n), f8)
    c[:, 0] = l_lo * sh(u_lo, -1)
    c[:, 1] = l_lo * sh(u_dm, -1) + l_dm * u_lo
    c[:, 2] = l_lo * sh(u_hi, -1) + l_dm * u_dm + l_hi * sh(u_lo, +1)
    c[:, 3] = l_dm * u_hi + l_hi * sh(u_dm, +1)
    c[:, 4] = l_hi * sh(u_hi, +1)
    c[:, 0, 0:2] = 0
    c[:, 1, 0:1] = 0
    c[:, 3, n-1:] = 0
    c[:, 4, n-2:] = 0
    if transform:
        c *= Diag[:, None, :]
    else:
        for k in range(5):
            c[:, k] = c[:, k] * sh(Diag.astype(f8), k - 2)
    return c


def _block_plan():
    plan = []
    o0 = 0
    while o0 < N:
        mcount = min(MOUT, N - o0)
        w0 = min(max(o0 - 2, 0), N - KWIN)
        plan.append((o0, mcount, w0))
        o0 += mcount
    return plan


def _build_lhst(c, plan):
    """c: [B, 5, N] -> k-major slabs [B, KWIN, nslot*KWIN] (cols zero-padded).

    Slab j (cols j*KWIN .. j*KWIN+mcount) is block j's lhsT:
    lhst[b, r, j*KWIN + m] = c[b, (w0+r)-(o0+m)+2, o0+m] when in band."""
    nslot = len(plan)
    Bn = c.shape[0]
    lhst = np.zeros((Bn, nslot, KWIN, KWIN), np.float32)
    r = np.arange(KWIN)
    for j, (o0, mcount, w0) in enumerate(plan):
        m = np.arange(mcount)
        off = (w0 + r[:, None]) - (o0 + m[None, :])
        valid = (off >= -2) & (off <= 2)
        rr, mm = np.nonzero(valid)
        lhst[:, j, rr, mm] = c[:, off[rr, mm] + 2, o0 + mm].astype(np.float32)
    # k-major: [B, KWIN(k), nslot, KWIN(m)] flattened to [B, KWIN, nslot*KWIN]
    return np.ascontiguousarray(lhst.transpose(0, 2, 1, 3)).reshape(
        Bn, KWIN, nslot * KWIN)


def _store_groups(plan):
    nblk = len(plan)
    if CFG["gplan"]:
        sizes = list(CFG["gplan"])
        groups = []
        jj = 0
        for s in sizes:
            g = []
            while jj < nblk and plan[jj][1] == MOUT and len(g) < s:
                g.append(jj)
                jj += 1
            if g:
                groups.append(g)
        while jj < nblk:
            groups.append([jj])
            jj += 1
        return groups
    groups = []
    jj = 0
    while jj < nblk:
        g = []
        while jj < nblk and plan[jj][1] == MOUT and len(g) < CFG["GH"]:
            g.append(jj)
            jj += 1
        if not g:
            g = [jj]
            jj += 1
        groups.append(g)
    return groups


# ---------------------------------------------------------------- device program

def _build_program(transform, reps=1, strip=""):
    import concourse.bass as bass  # noqa: F401
    import concourse.mybir as mybir
    import concourse.tile as tile
    from concourse import bacc
    from concourse.ap import AP

    F32 = mybir.dt.float32
    DT = {"f32": mybir.dt.float32, "bf16": mybir.dt.bfloat16}[CFG["dt"]]
    plan = _block_plan()
    nblk = len(plan)
    nslot = nblk

    nc = bacc.Bacc(None, target_bir_lowering=False, num_swdge_queues=CFG["swq"])
    store_eng = {"gpsimd": nc.gpsimd, "scalar": nc.scalar, "sync": nc.sync,
                 "vector": nc.vector}[CFG["store_eng"]]
    x = nc.declare_dram_parameter("x", [N, D], DT, isOutput=False)
    lhst = nc.declare_dram_parameter("lhst", [KWIN, nslot * KWIN], DT,
                                     isOutput=False)
    z = nc.declare_dram_parameter("z", [N * D], DT, isOutput=True)

    # x-load chunks: affine same-stride runs of windows, split to <= XCH.
    # Blocks 0 and nblk-1 have clamped w0 that breaks the affine
    # progression, so they get their own chunks.
    XCH = CFG["XCH"]
    runs = [[0], list(range(1, nblk - 1)), [nblk - 1]]
    xchunks = []
    for r in runs:
        if len(r) == 1:
            xchunks.append(r)
        elif CFG["xplan"]:
            s = 0
            xp = list(CFG["xplan"])
            while s < len(r):
                w = xp.pop(0) if xp else XCH
                xchunks.append(r[s:s + w])
                s += w
        else:
            for s in range(0, len(r), XCH):
                xchunks.append(r[s:s + XCH])
    xchunk_of = {}
    for ci, chsub in enumerate(xchunks):
        for pos, j in enumerate(chsub):
            xchunk_of[j] = (ci, pos)

    # lhsT chunks of up to LCH slots
    LCH = CFG["LCH"]
    lchunk_of = {s: (s // LCH, s % LCH) for s in range(nslot)}

    groups = _store_groups(plan)

    with tile.TileContext(nc) as tc:
        with (
            tc.tile_pool(name="xg", bufs=CFG["xg_bufs"]) as xgpool,
            tc.tile_pool(name="lh", bufs=CFG["lh_bufs"]) as lhpool,
            tc.tile_pool(name="psum", bufs=CFG["psum_bufs"], space="PSUM") as pspool,
            tc.tile_pool(name="stage", bufs=CFG["stage_bufs"]) as stpool,
        ):
            state = {"ev": 0, "st": 0, "pq": 0}
            xg_tiles = {}
            lh_tiles = {}

            def ensure_xchunk(ci):
                if ci in xg_tiles:
                    return xg_tiles[ci]
                chsub = xchunks[ci]
                j0 = chsub[0]
                nwin = len(chsub)
                xt = xgpool.tile([KWIN, nwin * D], DT, tag="xg")
                if strip in ("noxload", "dmaonly_nox", "mmonly", "empty"):
                    nc.sync.dma_start(out=xt[:1, :1], in_=x[0:1, 0:1])
                else:
                    nc.sync.dma_start(
                        out=xt[:, :].rearrange("p (j d) -> p j d", d=D),
                        in_=AP(x, plan[j0][2] * D,
                               [[D, KWIN], [MOUT * D, nwin], [1, D]]),
                    )
                xg_tiles[ci] = xt
                return xt

            def ensure_lchunk(li):
                if li in lh_tiles:
                    return lh_tiles[li]
                s0 = li * LCH
                cnt = min(LCH, nslot - s0)
                lht = lhpool.tile([KWIN, cnt * KWIN], DT, tag="lh")
                lh_eng = {"sync": nc.sync, "scalar": nc.scalar,
                          "gpsimd": nc.gpsimd}[CFG["lh_eng"]]
                if strip in ("nolhst", "mmonly", "empty"):
                    lh_eng.dma_start(out=lht[:1, :1], in_=lhst[0:1, 0:1])
                else:
                    lh_eng.dma_start(
                        out=lht[:, :],
                        in_=lhst[:, s0 * KWIN:(s0 + cnt) * KWIN],
                    )
                lh_tiles[li] = lht
                return lht

            def emit_body():
                xg_tiles.clear()
                lh_tiles.clear()
                order = list(groups)
                if CFG["pfirst"] and len(order) > 1 and len(order[-1]) == 1:
                    order = [order[-1]] + order[:-1]
                for g in order:
                    emit_group(g)

            def emit_group(g):
                glen = len(g)
                full = all(plan[j][1] == MOUT for j in g)
                if full:
                    stg = stpool.tile([MOUT, glen * D], DT, tag="stage")
                for gi, j in enumerate(g):
                    o0, mcount, w0 = plan[j]
                    ps = pspool.tile([mcount, D], F32, tag="psum")
                    li, lpos = lchunk_of[j]
                    lht = ensure_lchunk(li)
                    lh_ap = lht[:, lpos * KWIN: lpos * KWIN + mcount]
                    ci, cpos = xchunk_of[j]
                    xt = ensure_xchunk(ci)
                    nomm = strip in ("nomm", "dmaonly_nox", "empty")
                    if not nomm:
                        nc.tensor.matmul(ps[:, :], lh_ap,
                                         xt[:, cpos * D:(cpos + 1) * D],
                                         start=True, stop=True)
                    # PSUM -> SBUF eviction, mostly DVE (ACT also issues)
                    dst = stg[:, gi * D:(gi + 1) * D] if full else None
                    if dst is None:
                        stg1 = stpool.tile([mcount, D], DT, tag="stage_s")
                        dst = stg1[:, :]
                    if not nomm:
                        evr = CFG["evr"]
                        if evr > 0 and state["ev"] % evr == evr - 1:
                            nc.scalar.copy(dst, ps[:, :])
                        else:
                            nc.vector.tensor_copy(dst, ps[:, :])
                    elif gi == 0:
                        nc.vector.memset(dst[:1, :1], 0.0)
                    state["ev"] += 1
                    if not full:
                        if strip in ("nostore", "mmonly", "empty"):
                            store_eng.dma_start(out=z[0:1], in_=stg1[:1, :1])
                        else:
                            store_eng.dma_start(
                                out=AP(z, o0 * D, [[D, mcount], [1, D]]),
                                in_=stg1[:, :])
                if full:
                    o0g = plan[g[0]][0]
                    if strip in ("nostore", "mmonly", "empty"):
                        store_eng.dma_start(out=z[0:1], in_=stg[:1, :1])
                    else:
                        # blocked layout: z[o0g*D + p*(glen*D) + gi*D + d],
                        # optionally sliced along partitions over several
                        # queues/engines for parallel DMA processing
                        nsp = CFG["ssplit"]
                        smix = CFG["smix"]
                        bounds = [round(MOUT * i / nsp) for i in range(nsp + 1)]
                        for si in range(nsp):
                            p0, p1 = bounds[si], bounds[si + 1]
                            if p0 == p1:
                                continue
                            kind = smix[(state["st"] + si) % len(smix)]
                            dst = AP(z, o0g * D + p0 * glen * D,
                                     [[glen * D, p1 - p0], [1, glen * D]])
                            src = stg[p0:p1, :]
                            if kind == "p":
                                nc.gpsimd.dma_start(out=dst, in_=src)
                            elif kind == "a":
                                nc.scalar.dma_start(out=dst, in_=src)
                            else:
                                nc.sync.dma_start(out=dst, in_=src)
                        state["st"] += 1

            if reps == 1:
                emit_body()
            elif reps < 0:
                for _ in range(-reps):
                    emit_body()
            else:
                with tc.For_i(0, reps, 1):
                    emit_body()
    nc.compile()
    return nc, plan, None, nslot


def _get_program(transform, reps=1, strip=""):
    key = (int(bool(transform)), reps, strip, tuple(sorted(CFG.items())))
    if key not in _prog_cache:
        _prog_cache[key] = _build_program(key[0], reps, strip)
    return _prog_cache[key]


def _bench_arrays():
    """Random device-shaped inputs for timing runs."""
    rng = np.random.default_rng(0)
    nslot = len(_block_plan())
    npdt = _np_dt()
    xa = (rng.standard_normal((N, D)) * 0.1).astype(npdt)
    la = (rng.standard_normal((KWIN, nslot * KWIN)) * 0.01).astype(npdt)
    return {"x": xa, "lhst": la}


def _unblock(zb, plan, groups):
    """zb: [B, N*D] blocked -> [B, N, D] natural (sweep-row) order."""
    out = np.empty((zb.shape[0], N, D), zb.dtype)
    for g in groups:
        glen = len(g)
        o0g = plan[g[0]][0]
        if all(plan[j][1] == MOUT for j in g):
            seg = zb[:, o0g * D:(o0g + MOUT * glen) * D]
            seg = seg.reshape(zb.shape[0], MOUT, glen, D).transpose(0, 2, 1, 3)
            out[:, o0g:o0g + MOUT * glen] = seg.reshape(zb.shape[0], -1, D)
        else:
            o0, mcount, _ = plan[g[0]]
            out[:, o0:o0 + mcount] = zb[:, o0 * D:(o0 + mcount) * D].reshape(
                zb.shape[0], mcount, D)
    return out


# ---------------------------------------------------------------- entry point

def kernel(input, G_l_ii, G_l_ij, G_l_ji, G_l_jj,
           G_u_ii, G_u_ij, G_u_ji, G_u_jj, Diag, transform, _run_kwargs=None):
    from concourse.bass_utils import run_bass_kernel_spmd

    transform = int(np.asarray(transform))
    npdt = _np_dt()
    x_full = np.asarray(input, dtype=np.float32)
    if transform:
        # sweep-row order: even rows then odd rows
        x_dev = np.concatenate([x_full[:, 0::2], x_full[:, 1::2]], axis=1)
    else:
        x_dev = x_full
    x_dev = np.ascontiguousarray(x_dev).astype(npdt)

    nc, plan, _, nslot = _get_program(transform)
    c = _penta_coeffs(np.asarray(G_l_ii), np.asarray(G_l_ij), np.asarray(G_l_ji),
                      np.asarray(G_l_jj), np.asarray(G_u_ii), np.asarray(G_u_ij),
                      np.asarray(G_u_ji), np.asarray(G_u_jj), np.asarray(Diag),
                      transform)
    lhst = _build_lhst(c, plan).astype(npdt)

    in_maps = [
        {"x": x_dev[b], "lhst": np.ascontiguousarray(lhst[b])}
        for b in range(B)
    ]
    kw = dict(_run_kwargs or {})
    res = run_bass_kernel_spmd(nc, in_maps, list(range(NCORES)), **kw)
    zb = np.stack([np.asarray(res.results[b]["z"]) for b in range(B)], axis=0)
    out = _unblock(zb.astype(np.float32), plan, _store_groups(plan))
    if not transform:
        # store-side stride permutation done on host for the untransformed path
        out = np.concatenate([out[:, 0::2], out[:, 1::2]], axis=1)
    out = np.ascontiguousarray(out.astype(np.float32, copy=False))
    if _run_kwargs is not None:
        return out, res
    return out


# revision 22
# speedup vs baseline: 1.5566x; 1.5566x over previous
"""Trainium2 Bass kernel for nn_DHHPTransform.

The reference op is: optional stride-2 permutation along N, an upper
tridiagonal Givens sweep, a lower tridiagonal sweep, and a diagonal
scale.  The two sweeps compose into a single *pentadiagonal* operator
  z[i] = sum_{k=-2..2} c_k[i] * x[i+k]
whose coefficients c_k (and the Diag fold) are O(B*N) and precomputed
on host.  The device kernel is a banded matvec: for each 128-row input
window one bf16 matmul  out[124, 256] = lhsT[128, 124].T @ win, PSUM
evicted to SBUF bf16, grouped contiguous stores.

All device-side IO is CONTIGUOUS in bf16:
 - x is pre-permuted on host (sweep-row order), so window loads are
   plain slices;
 - the banded slabs are host-baked k-major [KWIN, nslot*KWIN] so chunk
   loads are per-partition contiguous;
 - z is stored in a blocked [p][block][d] layout (one linear range per
   store group) and unscrambled on host.

Sharding: pure data-parallel, one batch element per NeuronCore.
"""

import numpy as np

B, N, D = 8, 8192, 256
KWIN = 128           # matmul contraction window (input rows per block)
MOUT = KWIN - 4      # output rows per block (window = out rows +2 halo each side)
NCORES = 8

# tunables; _get_program cache key includes them
CFG = {"XCH": 22, "LCH": 17, "GH": 11, "store_eng": "gpsimd", "psum_bufs": 6,
       "xg_bufs": 3, "stage_bufs": 4, "lh_bufs": 3, "swq": 1, "evr": 3,
       "dt": "bf16", "ssplit": 1, "smix": "p", "gplan": (), "xplan": (),
       "pfirst": 0, "lh_eng": "gpsimd", "x_alt": 0}

_prog_cache = {}


def _np_dt():
    if CFG["dt"] == "f32":
        return np.float32
    import ml_dtypes
    return ml_dtypes.bfloat16


# ---------------------------------------------------------------- host math

def _penta_coeffs(G_l_ii, G_l_ij, G_l_ji, G_l_jj,
                  G_u_ii, G_u_ij, G_u_ji, G_u_jj, Diag, transform):
    """[B, 5, N] pentadiagonal coefficients; index k means offset k-2."""
    Bn, n = Diag.shape
    f8 = np.float64
    u_lo = np.zeros((Bn, n), f8); u_dm = np.zeros((Bn, n), f8); u_hi = np.zeros((Bn, n), f8)
    u_dm[:, 0] = G_u_ii[:, 0]
    u_hi[:, 0] = G_u_ij[:, 0]
    u_lo[:, 1:n-1] = G_u_ji[:, :-1]
    u_dm[:, 1:n-1] = G_u_jj[:, :-1].astype(f8) * G_u_ii[:, 1:]
    u_hi[:, 1:n-1] = G_u_jj[:, :-1].astype(f8) * G_u_ij[:, 1:]
    u_lo[:, n-1] = G_u_ji[:, n-2]
    u_dm[:, n-1] = G_u_jj[:, n-2]
    l_lo = np.zeros((Bn, n), f8); l_dm = np.zeros((Bn, n), f8); l_hi = np.zeros((Bn, n), f8)
    l_dm[:, 0] = G_l_ii[:, 0]
    l_hi[:, 0] = G_l_ij[:, 0]
    l_lo[:, 1:n-1] = G_l_ii[:, 1:n-1].astype(f8) * G_l_ji[:, :n-2]
    l_dm[:, 1:n-1] = G_l_ii[:, 1:n-1].astype(f8) * G_l_jj[:, :n-2]
    l_hi[:, 1:n-1] = G_l_ij[:, 1:n-1]
    l_lo[:, n-1] = G_l_ji[:, n-2]
    l_dm[:, n-1] = G_l_jj[:, n-2]

    def sh(a, k):
        out = np.zeros_like(a)
        if k == 0:
            return a.copy()
        if k > 0:
            out[:, :-k] = a[:, k:]
        else:
            out[:, -k:] = a[:, :k]
        return out

    c = np.zeros((Bn, 5, n), f8)
    c[:, 0] = l_lo * sh(u_lo, -1)
    c[:, 1] = l_lo * sh(u_dm, -1) + l_dm * u_lo
    c[:, 2] = l_lo * sh(u_hi, -1) + l_dm * u_dm + l_hi * sh(u_lo, +1)
    c[:, 3] = l_dm * u_hi + l_hi * sh(u_dm, +1)
    c[:, 4] = l_hi * sh(u_hi, +1)
    c[:, 0, 0:2] = 0
    c[:, 1, 0:1] = 0
    c[:, 3, n-1:] = 0
    c[:, 4, n-2:] = 0
    if transform:
        c *= Diag[:, None, :]
    else:
        for k in range(5):
            c[:, k] = c[:, k] * sh(Diag.astype(f8), k - 2)
    return c


def _block_plan():
    plan = []
    o0 = 0
    while o0 < N:
        mcount = min(MOUT, N - o0)
        w0 = min(max(o0 - 2, 0), N - KWIN)
        plan.append((o0, mcount, w0))
        o0 += mcount
    return plan


def _build_lhst(c, plan):
    """c: [B, 5, N] -> k-major slabs [B, KWIN, nslot*KWIN] (cols zero-padded).

    Slab j (cols j*KWIN .. j*KWIN+mcount) is block j's lhsT:
    lhst[b, r, j*KWIN + m] = c[b, (w0+r)-(o0+m)+2, o0+m] when in band."""
    nslot = len(plan)
    Bn = c.shape[0]
    lhst = np.zeros((Bn, nslot, KWIN, KWIN), np.float32)
    r = np.arange(KWIN)
    for j, (o0, mcount, w0) in enumerate(plan):
        m = np.arange(mcount)
        off = (w0 + r[:, None]) - (o0 + m[None, :])
        valid = (off >= -2) & (off <= 2)
        rr, mm = np.nonzero(valid)
        lhst[:, j, rr, mm] = c[:, off[rr, mm] + 2, o0 + mm].astype(np.float32)
    # k-major: [B, KWIN(k), nslot, KWIN(m)] flattened to [B, KWIN, nslot*KWIN]
    return np.ascontiguousarray(lhst.transpose(0, 2, 1, 3)).reshape(
        Bn, KWIN, nslot * KWIN)


def _store_groups(plan):
    nblk = len(plan)
    if CFG["gplan"]:
        sizes = list(CFG["gplan"])
        groups = []
        jj = 0
        for s in sizes:
            g = []
            while jj < nblk and plan[jj][1] == MOUT and len(g) < s:
                g.append(jj)
                jj += 1
            if g:
                groups.append(g)
        while jj < nblk:
            groups.append([jj])
            jj += 1
        return groups
    groups = []
    jj = 0
    while jj < nblk:
        g = []
        while jj < nblk and plan[jj][1] == MOUT and len(g) < CFG["GH"]:
            g.append(jj)
            jj += 1
        if not g:
            g = [jj]
            jj += 1
        groups.append(g)
    return groups


# ---------------------------------------------------------------- device program

def _build_program(transform, reps=1, strip=""):
    import concourse.bass as bass  # noqa: F401
    import concourse.mybir as mybir
    import concourse.tile as tile
    from concourse import bacc
    from concourse.ap import AP

    F32 = mybir.dt.float32
    DT = {"f32": mybir.dt.float32, "bf16": mybir.dt.bfloat16}[CFG["dt"]]
    plan = _block_plan()
    nblk = len(plan)
    nslot = nblk

    nc = bacc.Bacc(None, target_bir_lowering=False, num_swdge_queues=CFG["swq"])
    store_eng = {"gpsimd": nc.gpsimd, "scalar": nc.scalar, "sync": nc.sync,
                 "vector": nc.vector}[CFG["store_eng"]]
    x = nc.declare_dram_parameter("x", [N, D], DT, isOutput=False)
    lhst = nc.declare_dram_parameter("lhst", [KWIN, nslot * KWIN], DT,
                                     isOutput=False)
    z = nc.declare_dram_parameter("z", [N * D], DT, isOutput=True)

    # x-load chunks: affine same-stride runs of windows, split to <= XCH.
    # Blocks 0 and nblk-1 have clamped w0 that breaks the affine
    # progression, so they get their own chunks.
    XCH = CFG["XCH"]
    runs = [[0], list(range(1, nblk - 1)), [nblk - 1]]
    xchunks = []
    for r in runs:
        if len(r) == 1:
            xchunks.append(r)
        elif CFG["xplan"]:
            s = 0
            xp = list(CFG["xplan"])
            while s < len(r):
                w = xp.pop(0) if xp else XCH
                xchunks.append(r[s:s + w])
                s += w
        else:
            for s in range(0, len(r), XCH):
                xchunks.append(r[s:s + XCH])
    xchunk_of = {}
    for ci, chsub in enumerate(xchunks):
        for pos, j in enumerate(chsub):
            xchunk_of[j] = (ci, pos)

    # lhsT chunks of up to LCH slots
    LCH = CFG["LCH"]
    lchunk_of = {s: (s // LCH, s % LCH) for s in range(nslot)}

    groups = _store_groups(plan)

    with tile.TileContext(nc) as tc:
        with (
            tc.tile_pool(name="xg", bufs=CFG["xg_bufs"]) as xgpool,
            tc.tile_pool(name="lh", bufs=CFG["lh_bufs"]) as lhpool,
            tc.tile_pool(name="psum", bufs=CFG["psum_bufs"], space="PSUM") as pspool,
            tc.tile_pool(name="stage", bufs=CFG["stage_bufs"]) as stpool,
        ):
            state = {"ev": 0, "st": 0, "pq": 0}
            xg_tiles = {}
            lh_tiles = {}

            def ensure_xchunk(ci):
                if ci in xg_tiles:
                    return xg_tiles[ci]
                chsub = xchunks[ci]
                j0 = chsub[0]
                nwin = len(chsub)
                xt = xgpool.tile([KWIN, nwin * D], DT, tag="xg")
                x_eng = nc.scalar if (CFG["x_alt"] and ci % 2) else nc.sync
                if strip in ("noxload", "dmaonly_nox", "mmonly", "empty"):
                    x_eng.dma_start(out=xt[:1, :1], in_=x[0:1, 0:1])
                else:
                    x_eng.dma_start(
                        out=xt[:, :].rearrange("p (j d) -> p j d", d=D),
                        in_=AP(x, plan[j0][2] * D,
                               [[D, KWIN], [MOUT * D, nwin], [1, D]]),
                    )
                xg_tiles[ci] = xt
                return xt

            def ensure_lchunk(li):
                if li in lh_tiles:
                    return lh_tiles[li]
                s0 = li * LCH
                cnt = min(LCH, nslot - s0)
                lht = lhpool.tile([KWIN, cnt * KWIN], DT, tag="lh")
                lh_eng = {"sync": nc.sync, "scalar": nc.scalar,
                          "gpsimd": nc.gpsimd}[CFG["lh_eng"]]
                if strip in ("nolhst", "mmonly", "empty"):
                    lh_eng.dma_start(out=lht[:1, :1], in_=lhst[0:1, 0:1])
                else:
                    lh_eng.dma_start(
                        out=lht[:, :],
                        in_=lhst[:, s0 * KWIN:(s0 + cnt) * KWIN],
                    )
                lh_tiles[li] = lht
                return lht

            def emit_body():
                xg_tiles.clear()
                lh_tiles.clear()
                order = list(groups)
                if CFG["pfirst"] and len(order) > 1 and len(order[-1]) == 1:
                    order = [order[-1]] + order[:-1]
                for g in order:
                    emit_group(g)

            def emit_group(g):
                glen = len(g)
                full = all(plan[j][1] == MOUT for j in g)
                if full:
                    stg = stpool.tile([MOUT, glen * D], DT, tag="stage")
                for gi, j in enumerate(g):
                    o0, mcount, w0 = plan[j]
                    ps = pspool.tile([mcount, D], F32, tag="psum")
                    li, lpos = lchunk_of[j]
                    lht = ensure_lchunk(li)
                    lh_ap = lht[:, lpos * KWIN: lpos * KWIN + mcount]
                    ci, cpos = xchunk_of[j]
                    xt = ensure_xchunk(ci)
                    nomm = strip in ("nomm", "dmaonly_nox", "empty")
                    if not nomm:
                        nc.tensor.matmul(ps[:, :], lh_ap,
                                         xt[:, cpos * D:(cpos + 1) * D],
                                         start=True, stop=True)
                    # PSUM -> SBUF eviction, mostly DVE (ACT also issues)
                    dst = stg[:, gi * D:(gi + 1) * D] if full else None
                    if dst is None:
                        stg1 = stpool.tile([mcount, D], DT, tag="stage_s")
                        dst = stg1[:, :]
                    if not nomm:
                        evr = CFG["evr"]
                        if evr > 0 and state["ev"] % evr == evr - 1:
                            nc.scalar.copy(dst, ps[:, :])
                        else:
                            nc.vector.tensor_copy(dst, ps[:, :])
                    elif gi == 0:
                        nc.vector.memset(dst[:1, :1], 0.0)
                    state["ev"] += 1
                    if not full:
                        if strip in ("nostore", "mmonly", "empty"):
                            store_eng.dma_start(out=z[0:1], in_=stg1[:1, :1])
                        else:
                            store_eng.dma_start(
                                out=AP(z, o0 * D, [[D, mcount], [1, D]]),
                                in_=stg1[:, :])
                if full:
                    o0g = plan[g[0]][0]
                    if strip in ("nostore", "mmonly", "empty"):
                        store_eng.dma_start(out=z[0:1], in_=stg[:1, :1])
                    else:
                        # blocked layout: z[o0g*D + p*(glen*D) + gi*D + d],
                        # optionally sliced along partitions over several
                        # queues/engines for parallel DMA processing
                        nsp = CFG["ssplit"]
                        smix = CFG["smix"]
                        bounds = [round(MOUT * i / nsp) for i in range(nsp + 1)]
                        for si in range(nsp):
                            p0, p1 = bounds[si], bounds[si + 1]
                            if p0 == p1:
                                continue
                            kind = smix[(state["st"] + si) % len(smix)]
                            dst = AP(z, o0g * D + p0 * glen * D,
                                     [[glen * D, p1 - p0], [1, glen * D]])
                            src = stg[p0:p1, :]
                            if kind == "p":
                                nc.gpsimd.dma_start(out=dst, in_=src)
                            elif kind == "a":
                                nc.scalar.dma_start(out=dst, in_=src)
                            else:
                                nc.sync.dma_start(out=dst, in_=src)
                        state["st"] += 1

            if reps == 1:
                emit_body()
            elif reps < 0:
                for _ in range(-reps):
                    emit_body()
            else:
                with tc.For_i(0, reps, 1):
                    emit_body()
    nc.compile()
    return nc, plan, None, nslot


def _get_program(transform, reps=1, strip=""):
    key = (int(bool(transform)), reps, strip, tuple(sorted(CFG.items())))
    if key not in _prog_cache:
        _prog_cache[key] = _build_program(key[0], reps, strip)
    return _prog_cache[key]


def _bench_arrays():
    """Random device-shaped inputs for timing runs."""
    rng = np.random.default_rng(0)
    nslot = len(_block_plan())
    npdt = _np_dt()
    xa = (rng.standard_normal((N, D)) * 0.1).astype(npdt)
    la = (rng.standard_normal((KWIN, nslot * KWIN)) * 0.01).astype(npdt)
    return {"x": xa, "lhst": la}


def _unblock(zb, plan, groups):
    """zb: [B, N*D] blocked -> [B, N, D] natural (sweep-row) order."""
    out = np.empty((zb.shape[0], N, D), zb.dtype)
    for g in groups:
        glen = len(g)
        o0g = plan[g[0]][0]
        if all(plan[j][1] == MOUT for j in g):
            seg = zb[:, o0g * D:(o0g + MOUT * glen) * D]
            seg = seg.reshape(zb.shape[0], MOUT, glen, D).transpose(0, 2, 1, 3)
            out[:, o0g:o0g + MOUT * glen] = seg.reshape(zb.shape[0], -1, D)
        else:
            o0, mcount, _ = plan[g[0]]
            out[:, o0:o0 + mcount] = zb[:, o0 * D:(o0 + mcount) * D].reshape(
                zb.shape[0], mcount, D)
    return out


# ---------------------------------------------------------------- entry point

def kernel(input, G_l_ii, G_l_ij, G_l_ji, G_l_jj,
           G_u_ii, G_u_ij, G_u_ji, G_u_jj, Diag, transform, _run_kwargs=None):
    from concourse.bass_utils import run_bass_kernel_spmd

    transform = int(np.asarray(transform))
    npdt = _np_dt()
    x_full = np.asarray(input, dtype=np.float32)
    if transform:
        # sweep-row order: even rows then odd rows
        x_dev = np.concatenate([x_full[:, 0::2], x_full[:, 1::2]], axis=1)
    else:
        x_dev = x_full
    x_dev = np.ascontiguousarray(x_dev).astype(npdt)

    nc, plan, _, nslot = _get_program(transform)
    c = _penta_coeffs(np.asarray(G_l_ii), np.asarray(G_l_ij), np.asarray(G_l_ji),
                      np.asarray(G_l_jj), np.asarray(G_u_ii), np.asarray(G_u_ij),
                      np.asarray(G_u_ji), np.asarray(G_u_jj), np.asarray(Diag),
                      transform)
    lhst = _build_lhst(c, plan).astype(npdt)

    in_maps = [
        {"x": x_dev[b], "lhst": np.ascontiguousarray(lhst[b])}
        for b in range(B)
    ]
    kw = dict(_run_kwargs or {})
    res = run_bass_kernel_spmd(nc, in_maps, list(range(NCORES)), **kw)
    zb = np.stack([np.asarray(res.results[b]["z"]) for b in range(B)], axis=0)
    out = _unblock(zb.astype(np.float32), plan, _store_groups(plan))
    if not transform:
        # store-side stride permutation done on host for the untransformed path
        out = np.concatenate([out[:, 0::2], out[:, 1::2]], axis=1)
    out = np.ascontiguousarray(out.astype(np.float32, copy=False))
    if _run_kwargs is not None:
        return out, res
    return out
